# revision 1
# baseline (speedup 1.0000x reference)
"""Multi-head causal attention with RoPE on 8 trn2 cores.

Sharding: core c -> batch b = c // 4, head group g = c % 4 (heads 4g..4g+4).
Each core computes q/k/v projections for its 4 heads, causal attention, and
a partial output-projection (its heads' slice of Wo). The host sums the 4
partials per batch (tensor-parallel unshard) and adds the output bias.

Device layout notes:
  - x is passed transposed with a ones row appended: xT [1025, 2048] bf16.
  - Wq/Wk columns are permuted so the rotary "x1" halves of all 4 heads form
    output partitions 0..127 and the "x2" halves partitions 0..127 of a
    second chunk; RoPE is then 6 full-width vector ops per projection.
  - q/k are produced directly transposed ([d, s]); scores are computed
    transposed ([kk, q]) so the PV matmul consumes them as weights without
    any transpose, and the softmax denominator comes from a ones column
    appended to v (generated by the bias row of Wv).
  - The attention output is normalized with per-partition scalars, moved to
    [dh, s] layout via DMA xbar transposes, and hits the Wo matmul which
    writes the final output transposed ([fo, s]); the host transposes back.
"""

import os

import numpy as np
import ml_dtypes

BF16 = ml_dtypes.bfloat16

B, S, F = 2, 2048, 1024
H, D = 16, 64
HALF = D // 2
NCORES = 8
HPC = 4  # heads per core
S_TILES = S // 128  # 16
N_CH = S // 512  # 4  (512-wide column chunks of s)
F_CH = F // 128  # 8
MACROS = 4  # q macro tiles of 512
MAX_WAVELENGTH = 10000.0

_CACHE = {}
LAST_RESULT = None


def _build_nc():
    import concourse.bacc as bacc
    import concourse.tile as tile
    import concourse.mybir as mybir
    import concourse.bass as bass

    fp32 = mybir.dt.float32
    bf16 = mybir.dt.bfloat16
    MULT = mybir.AluOpType.mult
    ADD = mybir.AluOpType.add
    EXP = mybir.ActivationFunctionType.Exp
    IDENT = mybir.ActivationFunctionType.Identity

    nc = bacc.Bacc("TRN2", target_bir_lowering=False, debug=False)

    xT_d = nc.dram_tensor("xT", [F + 1, S], bf16, kind="ExternalInput")
    wq_d = nc.dram_tensor("wq", [F, 256], bf16, kind="ExternalInput")
    wk_d = nc.dram_tensor("wk", [F, 256], bf16, kind="ExternalInput")
    wv_d = nc.dram_tensor("wv", [F + 1, 260], bf16, kind="ExternalInput")
    wo_d = nc.dram_tensor("wo", [256, F], bf16, kind="ExternalInput")
    bqk_d = nc.dram_tensor("bqk", [128, 4], fp32, kind="ExternalInput")
    cos_d = nc.dram_tensor("cosw", [128, S], bf16, kind="ExternalInput")
    sin_d = nc.dram_tensor("sinw", [128, S], bf16, kind="ExternalInput")
    mask_d = nc.dram_tensor("mask", [128, 256], bf16, kind="ExternalInput")
    outT_d = nc.dram_tensor("outT", [F, S], fp32, kind="ExternalOutput")

    with tile.TileContext(nc) as tc:
        with (
            tc.tile_pool(name="persist", bufs=1) as persist,
            tc.tile_pool(name="tmp", bufs=8) as tmp,
            tc.tile_pool(name="attn", bufs=6) as attn_pool,
            tc.tile_pool(name="ostage", bufs=8) as ostage,
            tc.tile_pool(name="psA", bufs=2, space="PSUM") as psA,
            tc.tile_pool(name="psPV", bufs=4, space="PSUM") as psPV,
        ):
            # ---------------- persistent SBUF tensors + loads ----------
            xT = [persist.tile([128, S], bf16, tag=f"xT{i}", name=f"xT{i}") for i in range(F_CH)]
            xones = persist.tile([1, S], bf16, tag="xones", name="xones")
            for i in range(F_CH):
                nc.sync.dma_start(out=xT[i], in_=xT_d[128 * i : 128 * (i + 1), :])
            nc.sync.dma_start(out=xones, in_=xT_d[F : F + 1, :])

            wq = [persist.tile([128, 256], bf16, tag=f"wq{i}", name=f"wq{i}") for i in range(F_CH)]
            wk = [persist.tile([128, 256], bf16, tag=f"wk{i}", name=f"wk{i}") for i in range(F_CH)]
            wv = [persist.tile([128, 260], bf16, tag=f"wv{i}", name=f"wv{i}") for i in range(F_CH)]
            wvb = persist.tile([1, 260], bf16, tag="wvb", name="wvb")
            for i in range(F_CH):
                nc.sync.dma_start(out=wq[i], in_=wq_d[128 * i : 128 * (i + 1), :])
                nc.sync.dma_start(out=wk[i], in_=wk_d[128 * i : 128 * (i + 1), :])
                nc.sync.dma_start(out=wv[i], in_=wv_d[128 * i : 128 * (i + 1), :])
            nc.sync.dma_start(out=wvb, in_=wv_d[F : F + 1, :])

            wo = [persist.tile([128, F], bf16, tag=f"wo{i}", name=f"wo{i}") for i in range(2)]
            for i in range(2):
                nc.sync.dma_start(out=wo[i], in_=wo_d[128 * i : 128 * (i + 1), :])

            bqk = persist.tile([128, 4], fp32, tag="bqk", name="bqk")
            cosw = persist.tile([128, S], bf16, tag="cosw", name="cosw")
            sinw = persist.tile([128, S], bf16, tag="sinw", name="sinw")
            maskt = persist.tile([128, 256], bf16, tag="maskt", name="maskt")
            nc.sync.dma_start(out=bqk, in_=bqk_d[:, :])
            nc.sync.dma_start(out=cosw, in_=cos_d[:, :])
            nc.sync.dma_start(out=sinw, in_=sin_d[:, :])
            nc.sync.dma_start(out=maskt, in_=mask_d[:, :])

            # post-RoPE q/k, transposed layout [d, s]; chunk 0 = x1 halves
            # of the 4 heads (head h -> partitions 32h..32h+32), chunk 1 = x2.
            q1 = persist.tile([128, S], bf16, tag="q1", name="q1")
            q2 = persist.tile([128, S], bf16, tag="q2", name="q2")
            k1 = persist.tile([128, S], bf16, tag="k1", name="k1")
            k2 = persist.tile([128, S], bf16, tag="k2", name="k2")
            # v in [s, d] layout; head h cols 65h..65h+64, col 65h+64 = ones
            v_sb = [persist.tile([128, 260], bf16, tag=f"v{i}", name=f"v{i}") for i in range(S_TILES)]
            # attention output, [dh, s] layout (head h -> tile h//2 rows 64*(h%2))
            aoT = [persist.tile([128, S], bf16, tag=f"aoT{i}", name=f"aoT{i}") for i in range(2)]

            # ---------------- v projection ------------------------------
            for st in range(S_TILES):
                ps = psA.tile([128, 260], fp32, tag="sc", name="psv")
                sl = slice(128 * st, 128 * (st + 1))
                for kc in range(F_CH):
                    nc.tensor.matmul(ps, xT[kc][:, sl], wv[kc], start=(kc == 0), stop=False)
                nc.tensor.matmul(ps, xones[:, sl], wvb, start=False, stop=True)
                nc.vector.tensor_copy(v_sb[st], ps)

            # ---------------- q/k projections + RoPE --------------------
            for (w_sb, b0, o1, o2) in ((wq, 0, q1, q2), (wk, 2, k1, k2)):
                for n in range(N_CH):
                    nsl = slice(512 * n, 512 * (n + 1))
                    ps1 = psA.tile([128, 512], fp32, tag="sc", name="ps1")
                    ps2 = psA.tile([128, 512], fp32, tag="sc", name="ps2")
                    for kc in range(F_CH):
                        nc.tensor.matmul(ps1, w_sb[kc][:, 0:128], xT[kc][:, nsl],
                                         start=(kc == 0), stop=(kc == F_CH - 1))
                    for kc in range(F_CH):
                        nc.tensor.matmul(ps2, w_sb[kc][:, 128:256], xT[kc][:, nsl],
                                         start=(kc == 0), stop=(kc == F_CH - 1))
                    # drain psum via ACT copies with the bias fused in, then
                    # RoPE as bf16 tensor-tensor ops (2x DVE mode) in SBUF
                    c1 = tmp.tile([128, 512], bf16, tag="rope", name="c1")
                    c2 = tmp.tile([128, 512], bf16, tag="rope", name="c2")
                    nc.scalar.activation(c1, ps1, func=IDENT, bias=bqk[:, b0:b0 + 1])
                    nc.scalar.activation(c2, ps2, func=IDENT, bias=bqk[:, b0 + 1:b0 + 2])
                    t1 = tmp.tile([128, 512], bf16, tag="rope", name="t1")
                    t2 = tmp.tile([128, 512], bf16, tag="rope", name="t2")
                    t3 = tmp.tile([128, 512], bf16, tag="rope", name="t3")
                    t4 = tmp.tile([128, 512], bf16, tag="rope", name="t4")
                    # x1' = x1*cos - x2*sin ; x2' = x2*cos + x1*sin
                    nc.vector.tensor_mul(t1, c1, cosw[:, nsl])
                    nc.vector.tensor_mul(t2, c2, sinw[:, nsl])
                    nc.vector.tensor_mul(t3, c2, cosw[:, nsl])
                    nc.vector.tensor_mul(t4, c1, sinw[:, nsl])
                    nc.vector.tensor_sub(o1[:, nsl], t1, t2)
                    nc.vector.tensor_add(o2[:, nsl], t3, t4)

            # ---------------- attention ---------------------------------
            # scores transposed [kk, q]; two heads share one [128, 1024]
            # psum tile (head pair p: cols 512*hh); exp is one strided ACT op
            # over both heads' causal-valid columns. PV uses v as stationary:
            # out pvT[h] = [65 dh, 512 q] accumulated over kk, row 64 = sums.
            for m in range(MACROS):
                msl = slice(512 * m, 512 * (m + 1))
                pvT = [psPV.tile([65, 512], fp32, tag="pvT", name="pvT") for _ in range(HPC)]
                for kk in range(4 * m + 4):
                    t = kk - 4 * m  # >= 0 -> this kk-chunk holds the diagonal
                    lo = max(0, t) * 128
                    ksl = slice(128 * kk, 128 * (kk + 1))
                    pair_exp = os.environ.get("KVAR_PAIR_EXP", "1") == "1"
                    for p in range(2):
                        if pair_exp:
                            sps = psA.tile([128, 1024], fp32, tag="sc", name="sps")
                            for hh in range(2):
                                h = 2 * p + hh
                                hp = slice(32 * h, 32 * (h + 1))
                                tp = (32 * h, 0)
                                ssl = slice(512 * hh, 512 * hh + 512)
                                qsl = slice(512 * m + lo, 512 * (m + 1))
                                osl = slice(512 * hh + lo, 512 * hh + 512)
                                nc.tensor.matmul(sps[:, osl], k1[hp, ksl], q1[hp, qsl],
                                                 start=True, stop=False, tile_position=tp)
                                nc.tensor.matmul(sps[:, osl], k2[hp, ksl], q2[hp, qsl],
                                                 start=False, stop=True, tile_position=tp)
                            at = attn_pool.tile([128, 1024], bf16, tag="attn", name="at")
                            sps_v = sps[:, :].rearrange("a (h q) -> a h q", h=2)[:, :, lo:512]
                            at_v = at[:, :].rearrange("a (h q) -> a h q", h=2)[:, :, lo:512]
                            nc.scalar.activation(out=at_v, in_=sps_v, func=EXP, scale=0.125)
                            if t >= 0:
                                dv = at[:, :].rearrange("a (h q) -> a h q", h=2)[:, :, 128 * t:128 * (t + 1)]
                                nc.vector.tensor_tensor(dv, dv, maskt, op=MULT)
                            ats_p = [at[:, 0:512], at[:, 512:1024]]
                        else:
                            ats_p = []
                            for hh in range(2):
                                h = 2 * p + hh
                                hp = slice(32 * h, 32 * (h + 1))
                                tp = (32 * h, 0)
                                sps = psA.tile([128, 512], fp32, tag="sc", name="sps")
                                nc.tensor.matmul(sps, k1[hp, ksl], q1[hp, msl],
                                                 start=True, stop=False, tile_position=tp)
                                nc.tensor.matmul(sps, k2[hp, ksl], q2[hp, msl],
                                                 start=False, stop=True, tile_position=tp)
                                at = attn_pool.tile([128, 512], bf16, tag="attn", name="at")
                                nc.scalar.activation(out=at[:, lo:512], in_=sps[:, lo:512],
                                                     func=EXP, scale=0.125)
                                if t >= 0:
                                    dsl = slice(128 * t, 128 * (t + 1))
                                    nc.vector.tensor_tensor(at[:, dsl], at[:, dsl],
                                                            maskt[:, 0:128], op=MULT)
                                ats_p.append(at[:, 0:512])
                        for hh in range(2):
                            h = 2 * p + hh
                            nc.tensor.matmul(
                                pvT[h][:, lo:512],
                                v_sb[kk][:, 65 * h:65 * h + 65],
                                ats_p[hh][:, lo:512],
                                start=(kk == 0), stop=(kk == 4 * m + 3))
                # normalize: rows 0..63 scaled by 1/row64. HW constraints:
                # engine ops need partition base 0 on every operand, so the
                # sums row is extracted with a base-0 full copy, hopped to
                # partition 0 by DMA, reciprocal'd and broadcast at base 0,
                # and the final [dh, s] placement goes through DMA.
                for h in range(HPC):
                    cix, r0 = h // 2, 64 * (h % 2)
                    s65 = tmp.tile([65, 512], fp32, tag="s65", name="s65")
                    nc.vector.tensor_copy(s65, pvT[h][0:65, :])
                    rec0 = tmp.tile([1, 512], fp32, tag="rec0", name="rec0")
                    nc.sync.dma_start(out=rec0, in_=s65[64:65, :])
                    rcp = tmp.tile([1, 512], fp32, tag="rcp", name="rcp")
                    nc.vector.reciprocal_approx_fast(rcp, rec0)
                    rb = tmp.tile([64, 512], fp32, tag="rb", name="rb")
                    nc.gpsimd.partition_broadcast(rb, rcp[0:1, :])
                    ao = ostage.tile([64, 512], bf16, tag="ao", name="ao")
                    # read from s65 (not psum) so the pvT bank frees right
                    # after the copy and the next macro's PV can start
                    nc.vector.tensor_tensor(ao, s65[0:64, :], rb, op=MULT)
                    nc.sync.dma_start(out=aoT[cix][r0:r0 + 64, msl], in_=ao)

            # ---------------- output projection (transposed) ------------
            for fo in range(F_CH):
                fsl = slice(128 * fo, 128 * (fo + 1))
                for sc in range(N_CH):
                    pw = psA.tile([128, 512], fp32, tag="sc", name="pw")
                    for c in range(2):
                        nc.tensor.matmul(pw, wo[c][:, fsl],
                                         aoT[c][:, 512 * sc:512 * (sc + 1)],
                                         start=(c == 0), stop=(c == 1))
                    ow = ostage.tile([128, 512], fp32, tag="ow", name="ow")
                    if sc % 2 == 0:
                        nc.vector.tensor_copy(ow, pw)
                    else:
                        nc.scalar.copy(ow, pw)
                    nc.sync.dma_start(out=outT_d[fsl, 512 * sc:512 * (sc + 1)], in_=ow)

    nc.compile()
    return nc


def _get_nc():
    if "nc" not in _CACHE:
        _CACHE["nc"] = _build_nc()
    return _CACHE["nc"]


def _host_prep(x, positions, Wq, bq, Wk, bk, Wv, bv, Wo, bo):
    """Build the 8 per-core input maps."""
    ts = MAX_WAVELENGTH ** (2.0 * np.arange(HALF, dtype=np.float32) / D)  # [32]
    in_maps = []
    for c in range(NCORES):
        b, g = c // 4, c % 4
        heads = np.arange(4 * g, 4 * g + 4)
        cols_x1 = np.concatenate([64 * h + np.arange(32) for h in heads])
        cols_x2 = cols_x1 + 32
        perm = np.concatenate([cols_x1, cols_x2])

        xT = np.empty((F + 1, S), dtype=BF16)
        xT[:F] = x[b].T.astype(BF16)
        xT[F] = 1.0

        wv_e = np.zeros((F + 1, 260), dtype=np.float32)
        for hl, h in enumerate(heads):
            wv_e[:F, 65 * hl:65 * hl + 64] = Wv[:, 64 * h:64 * h + 64]
            wv_e[F, 65 * hl:65 * hl + 64] = bv[64 * h:64 * h + 64]
            wv_e[F, 65 * hl + 64] = 1.0

        bqk = np.stack([bq[cols_x1], bq[cols_x2], bk[cols_x1], bk[cols_x2]],
                       axis=1).astype(np.float32)  # [128, 4]

        pos = positions[b].astype(np.float32)  # [S]
        ang = pos[None, :] / ts[:, None]  # [32, S]
        cosw = np.tile(np.cos(ang), (4, 1)).astype(BF16)
        sinw = np.tile(np.sin(ang), (4, 1)).astype(BF16)

        ii = np.arange(128)
        mask = np.tile((ii[:, None] <= ii[None, :]).astype(BF16), (1, 2))

        in_maps.append({
            "xT": xT,
            "wq": Wq[:, perm].astype(BF16),
            "wk": Wk[:, perm].astype(BF16),
            "wv": wv_e.astype(BF16),
            "wo": Wo[64 * heads[0]:64 * heads[0] + 256, :].astype(BF16),
            "bqk": bqk,
            "cosw": cosw,
            "sinw": sinw,
            "mask": np.ascontiguousarray(mask),
        })
    return in_maps


def kernel(x, positions, Wq, bq, Wk, bk, Wv, bv, Wo, bo):
    global LAST_RESULT
    from concourse.bass_utils import run_bass_kernel_spmd

    x = np.asarray(x, dtype=np.float32)
    positions = np.asarray(positions)
    args = [np.asarray(a, dtype=np.float32) for a in (Wq, bq, Wk, bk, Wv, bv, Wo, bo)]
    Wq, bq, Wk, bk, Wv, bv, Wo, bo = args

    nc = _get_nc()
    in_maps = _host_prep(x, positions, Wq, bq, Wk, bk, Wv, bv, Wo, bo)
    try:
        res = run_bass_kernel_spmd(nc, in_maps, core_ids=list(range(NCORES)))
    except ModuleNotFoundError:
        # axon NTFF profiling hook unavailable in this image; run untraced
        os.environ["BASS_NEVER_TRACE"] = "1"
        res = run_bass_kernel_spmd(nc, in_maps, core_ids=list(range(NCORES)))
    LAST_RESULT = res

    out = np.empty((B, S, F), dtype=np.float32)
    for b in range(B):
        acc = np.zeros((F, S), dtype=np.float32)
        for g in range(4):
            acc += res.results[4 * b + g]["outT"]
        out[b] = acc.T + bo[None, :]
    return out



# revision 15
# speedup vs baseline: 1.2325x; 1.2325x over previous
"""Multi-head causal attention with RoPE on 8 trn2 cores.

Sharding: core c -> batch b = c // 4, head group g = c % 4 (heads 4g..4g+4).
Each core computes q/k/v projections for its 4 heads, causal attention, and
a partial output-projection (its heads' slice of Wo). The host sums the 4
partials per batch (tensor-parallel unshard) and adds the output bias.

v3 layout/schedule notes:
  - DMA order: wq first, then xT chunks, then wk/wv/rest, so projection
    matmuls start as soon as each xT chunk lands.
  - Wq/Wk columns are permuted so the rotary "x1" halves of all 4 heads form
    output partitions 0..127 and the "x2" halves a second 128 chunk; RoPE is
    6 full-width vector ops per projection chunk.
  - Attention runs per q-macro (512 q) in TWO PASSES of one head-pair each.
    Per kk chunk and pass: scoresT [kk, q] for the 2 heads go to a
    [128, 1024] psum pair-tile (bufs=2 -> next chunk's score matmuls overlap
    this chunk's EXP, keeping ScalarE's exp stream back-to-back). PV uses
    v with an appended ones column (M=65): psum row 64 accumulates the
    softmax denominator, one bank per head. Every psum bank holds exactly
    one matmul accumulation group covering one partition range.
  - Normalize: copy pv bank to SBUF (frees the bank), reciprocal of row 64
    read in place at partition base 64, gpsimd broadcast, one DVE multiply
    writing the pair-stacked attention output directly into aoT layout.
  - Projections for s-chunks 1..3 and the per-macro output projection are
    emitted between macros; they run on 2 dedicated psum banks ("lin",
    bufs=2) and fill TensorE gaps under the exp-bound attention phase.
    All psum drains stay off ScalarE during attention (DVE) so ScalarE
    does nothing but exp.
"""

import os

import numpy as np
import ml_dtypes

BF16 = ml_dtypes.bfloat16

B, S, F = 2, 2048, 1024
H, D = 16, 64
HALF = D // 2
NCORES = 8
HPC = 4  # heads per core
S_TILES = S // 128  # 16
N_CH = S // 512  # 4  (512-wide column chunks of s)
F_CH = F // 128  # 8
MACROS = 4  # q macro tiles of 512
MAX_WAVELENGTH = 10000.0

_CACHE = {}
LAST_RESULT = None


def _build_nc():
    import concourse.bacc as bacc
    import concourse.tile as tile
    import concourse.mybir as mybir

    fp32 = mybir.dt.float32
    bf16 = mybir.dt.bfloat16
    MULT = mybir.AluOpType.mult
    ADD = mybir.AluOpType.add
    EXP = mybir.ActivationFunctionType.Exp

    nc = bacc.Bacc("TRN2", target_bir_lowering=False, debug=False)

    xT_d = nc.dram_tensor("xT", [F, S], bf16, kind="ExternalInput")
    wq_d = nc.dram_tensor("wq", [F, 256], bf16, kind="ExternalInput")
    wk_d = nc.dram_tensor("wk", [F, 256], bf16, kind="ExternalInput")
    wv_d = nc.dram_tensor("wv", [F + 1, 260], bf16, kind="ExternalInput")
    wo_d = nc.dram_tensor("wo", [256, F], bf16, kind="ExternalInput")
    bqk_d = nc.dram_tensor("bqk", [128, 4], fp32, kind="ExternalInput")
    cos_d = nc.dram_tensor("cosw", [128, S], bf16, kind="ExternalInput")
    sin_d = nc.dram_tensor("sinw", [128, S], bf16, kind="ExternalInput")
    mask_d = nc.dram_tensor("mask", [128, 256], bf16, kind="ExternalInput")
    outT_d = nc.dram_tensor("outT", [F, S], fp32, kind="ExternalOutput")

    with tile.TileContext(nc) as tc:
        with (
            tc.tile_pool(name="persist", bufs=1) as persist,
            tc.tile_pool(name="tmp", bufs=8) as tmp,
            tc.tile_pool(name="attn", bufs=3) as attn_pool,
            tc.tile_pool(name="nrm", bufs=4) as nrm,
            tc.tile_pool(name="ostage", bufs=4) as ostage,
            tc.tile_pool(name="psSC", bufs=2, space="PSUM") as psSC,
            tc.tile_pool(name="psPV", bufs=2, space="PSUM") as psPV,
            tc.tile_pool(name="psLin", bufs=2, space="PSUM") as psLin,
        ):
            # ---------------- persistent SBUF tensors + loads ----------
            # load order = DMA issue order: wq, xT (q-proj can start), wk,
            # wv, then the small stuff and wo (needed last).
            bqk = persist.tile([128, 4], fp32, tag="bqk", name="bqk")
            nc.sync.dma_start(out=bqk, in_=bqk_d[:, :])
            wq = [persist.tile([128, 256], bf16, tag=f"wq{i}", name=f"wq{i}") for i in range(F_CH)]
            for i in range(F_CH):
                nc.sync.dma_start(out=wq[i], in_=wq_d[128 * i : 128 * (i + 1), :])
            xT = [persist.tile([128, S], bf16, tag=f"xT{i}", name=f"xT{i}") for i in range(F_CH)]
            for i in range(F_CH):
                nc.sync.dma_start(out=xT[i], in_=xT_d[128 * i : 128 * (i + 1), :])
            wk = [persist.tile([128, 256], bf16, tag=f"wk{i}", name=f"wk{i}") for i in range(F_CH)]
            for i in range(F_CH):
                nc.sync.dma_start(out=wk[i], in_=wk_d[128 * i : 128 * (i + 1), :])
            wv = [persist.tile([128, 260], bf16, tag=f"wv{i}", name=f"wv{i}") for i in range(F_CH)]
            for i in range(F_CH):
                nc.sync.dma_start(out=wv[i], in_=wv_d[128 * i : 128 * (i + 1), :])
            wvb = persist.tile([1, 260], bf16, tag="wvb", name="wvb")
            nc.sync.dma_start(out=wvb, in_=wv_d[F : F + 1, :])
            cosw = persist.tile([128, S], bf16, tag="cosw", name="cosw")
            sinw = persist.tile([128, S], bf16, tag="sinw", name="sinw")
            nc.sync.dma_start(out=cosw, in_=cos_d[:, :])
            nc.sync.dma_start(out=sinw, in_=sin_d[:, :])
            maskt = persist.tile([128, 256], bf16, tag="maskt", name="maskt")
            nc.sync.dma_start(out=maskt, in_=mask_d[:, :])
            wo = [persist.tile([128, F], bf16, tag=f"wo{i}", name=f"wo{i}") for i in range(2)]
            for i in range(2):
                nc.sync.dma_start(out=wo[i], in_=wo_d[128 * i : 128 * (i + 1), :])

            # post-RoPE q/k, transposed layout [d, s]; chunk 1 = x1 halves
            # of the 4 heads (head h -> partitions 32h..32h+32), chunk 2 = x2.
            q1 = persist.tile([128, S], bf16, tag="q1", name="q1")
            q2 = persist.tile([128, S], bf16, tag="q2", name="q2")
            k1 = persist.tile([128, S], bf16, tag="k1", name="k1")
            k2 = persist.tile([128, S], bf16, tag="k2", name="k2")
            # v in [s, d] layout; head h cols 65h..65h+64, col 65h+64 = ones
            v_sb = [persist.tile([128, 260], bf16, tag=f"v{i}", name=f"v{i}") for i in range(S_TILES)]
            # attention output, [dh, s] pair layout: pair p tile, head 2p at
            # rows 0..63, head 2p+1 at rows 64..127
            aoT = [persist.tile([128, S], bf16, tag=f"aoT{i}", name=f"aoT{i}") for i in range(2)]

            def emit_qk_half(n, w_sb, half, b0, out):
                # one projection half: 8 accumulating matmuls + DVE bias drain
                nsl = slice(512 * n, 512 * (n + 1))
                ps = psLin.tile([128, 512], fp32, tag="lin", name="ps")
                for kc in range(F_CH):
                    nc.tensor.matmul(ps, w_sb[kc][:, 128 * half:128 * half + 128],
                                     xT[kc][:, nsl],
                                     start=(kc == 0), stop=(kc == F_CH - 1))
                nc.vector.tensor_scalar_add(out, ps, bqk[:, b0 + half:b0 + half + 1])

            def emit_rope(n, c1, c2, o1, o2):
                nsl = slice(512 * n, 512 * (n + 1))
                t1 = tmp.tile([128, 512], bf16, tag="rope", name="t1")
                t2 = tmp.tile([128, 512], bf16, tag="rope", name="t2")
                t3 = tmp.tile([128, 512], bf16, tag="rope", name="t3")
                t4 = tmp.tile([128, 512], bf16, tag="rope", name="t4")
                # x1' = x1*cos - x2*sin ; x2' = x2*cos + x1*sin
                nc.vector.tensor_mul(t1, c1, cosw[:, nsl])
                nc.vector.tensor_mul(t2, c2, sinw[:, nsl])
                nc.vector.tensor_mul(t3, c2, cosw[:, nsl])
                nc.vector.tensor_mul(t4, c1, sinw[:, nsl])
                nc.vector.tensor_sub(o1[:, nsl], t1, t2)
                nc.vector.tensor_add(o2[:, nsl], t3, t4)

            def proj_fillers(n):
                # closures, each one psum-group, to interleave between
                # attention chunks (fills PE while ScalarE runs exp)
                fs = []
                for (w_sb, b0, o1, o2) in ((wq, 0, q1, q2), (wk, 2, k1, k2)):
                    c1 = tmp.tile([128, 512], bf16, tag="rope", name="c1")
                    c2 = tmp.tile([128, 512], bf16, tag="rope", name="c2")
                    fs.append(lambda n=n, w=w_sb, c=c1, b=b0: emit_qk_half(n, w, 0, b, c))
                    fs.append(lambda n=n, w=w_sb, c=c2, b=b0: emit_qk_half(n, w, 1, b, c))
                    fs.append(lambda n=n, a=c1, b=c2, u=o1, v=o2: emit_rope(n, a, b, u, v))
                for st in range(4 * n, 4 * n + 4):
                    fs.append(lambda st=st: emit_v_tile(st))
                return fs

            def emit_v_tile(st):
                # v projection for one s-tile; bias via the appended wv bias
                # row (K=1 matmul), drain on DVE
                ps = psLin.tile([128, 260], fp32, tag="lin", name="psv")
                sl = slice(128 * st, 128 * (st + 1))
                for kc in range(F_CH):
                    nc.tensor.matmul(ps, xT[kc][:, sl], wv[kc], start=(kc == 0), stop=False)
                nc.tensor.matmul(ps, xones[:, sl], wvb, start=False, stop=True)
                nc.vector.tensor_copy(v_sb[st], ps)

            def emit_attn_pass(m, p, fillers):
                # heads 2p, 2p+1 of q-macro m; pops one filler group after
                # each kk chunk so PE stays dense while ScalarE runs exp
                msl = slice(512 * m, 512 * (m + 1))
                pv = [psPV.tile([65, 512], fp32, tag="pv", name=f"pv{j}") for j in range(2)]
                last = 4 * m + 3
                for kk in range(4 * m + 4):
                    if fillers:
                        fillers.pop(0)()
                    t = kk - 4 * m  # >= 0 -> this kk-chunk holds the diagonal
                    lo = max(0, t) * 128
                    ksl = slice(128 * kk, 128 * (kk + 1))
                    qsl = slice(512 * m + lo, 512 * (m + 1))
                    sps = psSC.tile([128, 1024], fp32, tag="sc", name="sps")
                    for j in range(2):
                        h = 2 * p + j
                        hp = slice(32 * h, 32 * (h + 1))
                        osl = slice(512 * j + lo, 512 * j + 512)
                        nc.tensor.matmul(sps[:, osl], k1[hp, ksl], q1[hp, qsl],
                                         start=True, stop=False, tile_position=(32 * h, 0))
                    for j in range(2):
                        h = 2 * p + j
                        hp = slice(32 * h, 32 * (h + 1))
                        osl = slice(512 * j + lo, 512 * j + 512)
                        nc.tensor.matmul(sps[:, osl], k2[hp, ksl], q2[hp, qsl],
                                         start=False, stop=True, tile_position=(32 * h, 0))
                    at = attn_pool.tile([128, 1024], bf16, tag="attn", name="at")
                    sps_v = sps[:, :].rearrange("a (h q) -> a h q", h=2)[:, :, lo:512]
                    at_v = at[:, :].rearrange("a (h q) -> a h q", h=2)[:, :, lo:512]
                    nc.scalar.activation(out=at_v, in_=sps_v, func=EXP, scale=0.125)
                    if t >= 0:
                        dv = at[:, :].rearrange("a (h q) -> a h q", h=2)[:, :, 128 * t:128 * (t + 1)]
                        mv = maskt[:, :].rearrange("a (h q) -> a h q", h=2)
                        nc.vector.tensor_tensor(dv, dv, mv, op=MULT)
                    for j in range(2):
                        h = 2 * p + j
                        nc.tensor.matmul(
                            pv[j][:, lo:512],
                            v_sb[kk][:, 65 * h:65 * h + 65],
                            at[:, 512 * j + lo:512 * j + 512],
                            start=(kk == 0), stop=(kk == last))
                # normalize: rows 0..63 scaled by 1/row64. DVE ops require a
                # single base partition shared by ALL operands, so the sums
                # row is DMA-hopped to partition 0 and head 2p+1's output
                # goes through a base-0 staging tile + DMA into aoT.
                for j in range(2):
                    s65 = nrm.tile([65, 512], fp32, tag="s65", name="s65")
                    nc.vector.tensor_copy(s65, pv[j][0:65, :])
                    rcpin = nrm.tile([1, 512], fp32, tag="rcpin", name="rcpin")
                    nc.sync.dma_start(out=rcpin, in_=s65[64:65, :])
                    rcp = nrm.tile([1, 512], fp32, tag="rcp", name="rcp")
                    nc.vector.reciprocal_approx_fast(rcp, rcpin)
                    rb = nrm.tile([64, 512], fp32, tag="rb", name="rb")
                    nc.gpsimd.partition_broadcast(rb, rcp[0:1, :])
                    if j == 0:
                        nc.vector.tensor_tensor(aoT[p][0:64, msl],
                                                s65[0:64, :], rb, op=MULT)
                    else:
                        ao64 = nrm.tile([64, 512], bf16, tag="ao64", name="ao64")
                        nc.vector.tensor_tensor(ao64, s65[0:64, :], rb, op=MULT)
                        nc.sync.dma_start(out=aoT[p][64:128, msl], in_=ao64)

            def emit_out_chunk(m, fo):
                msl = slice(512 * m, 512 * (m + 1))
                fsl = slice(128 * fo, 128 * (fo + 1))
                pw = psLin.tile([128, 512], fp32, tag="lin", name="pw")
                for c in range(2):
                    nc.tensor.matmul(pw, wo[c][:, fsl], aoT[c][:, msl],
                                     start=(c == 0), stop=(c == 1))
                ow = ostage.tile([128, 512], fp32, tag="ow", name="ow")
                nc.vector.tensor_copy(ow, pw)
                nc.sync.dma_start(out=outT_d[fsl, msl], in_=ow)

            def out_proj_fillers(m):
                return [lambda m=m, fo=fo: emit_out_chunk(m, fo) for fo in range(F_CH)]

            # ones row of x for the v bias matmul
            xones = persist.tile([1, S], bf16, tag="xones", name="xones")
            nc.vector.memset(xones, 1.0)

            # ---------------- emission schedule ------------------------
            # proj(0) runs up front (gated by the input DMA stream); proj of
            # s-chunk m+1 and out-proj of macro m-1/m interleave as fillers
            # between the attention chunks of macro m.
            for f in proj_fillers(0):
                f()
            for m in range(MACROS):
                fillers = []
                if m + 1 < MACROS:
                    fillers += proj_fillers(m + 1)
                if m >= 1:
                    fillers += out_proj_fillers(m - 1)
                emit_attn_pass(m, 0, fillers)
                emit_attn_pass(m, 1, fillers)
                for f in fillers:  # leftovers before the next macro
                    f()
            for f in out_proj_fillers(MACROS - 1):
                f()

    nc.compile()
    return nc


def _get_nc():
    if "nc" not in _CACHE:
        _CACHE["nc"] = _build_nc()
    return _CACHE["nc"]


def _host_prep(x, positions, Wq, bq, Wk, bk, Wv, bv, Wo, bo):
    """Build the 8 per-core input maps."""
    ts = MAX_WAVELENGTH ** (2.0 * np.arange(HALF, dtype=np.float32) / D)  # [32]
    in_maps = []
    for c in range(NCORES):
        b, g = c // 4, c % 4
        heads = np.arange(4 * g, 4 * g + 4)
        cols_x1 = np.concatenate([64 * h + np.arange(32) for h in heads])
        cols_x2 = cols_x1 + 32
        perm = np.concatenate([cols_x1, cols_x2])

        wv_e = np.zeros((F + 1, 260), dtype=np.float32)
        for hl, h in enumerate(heads):
            wv_e[:F, 65 * hl:65 * hl + 64] = Wv[:, 64 * h:64 * h + 64]
            wv_e[F, 65 * hl:65 * hl + 64] = bv[64 * h:64 * h + 64]
            wv_e[F, 65 * hl + 64] = 1.0

        bqk = np.stack([bq[cols_x1], bq[cols_x2], bk[cols_x1], bk[cols_x2]],
                       axis=1).astype(np.float32)  # [128, 4]

        pos = positions[b].astype(np.float32)  # [S]
        ang = pos[None, :] / ts[:, None]  # [32, S]
        cosw = np.tile(np.cos(ang), (4, 1)).astype(BF16)
        sinw = np.tile(np.sin(ang), (4, 1)).astype(BF16)

        ii = np.arange(128)
        mask = np.tile((ii[:, None] <= ii[None, :]).astype(BF16), (1, 2))

        in_maps.append({
            "xT": np.ascontiguousarray(x[b].T).astype(BF16),
            "wq": Wq[:, perm].astype(BF16),
            "wk": Wk[:, perm].astype(BF16),
            "wv": wv_e.astype(BF16),
            "wo": Wo[64 * heads[0]:64 * heads[0] + 256, :].astype(BF16),
            "bqk": bqk,
            "cosw": cosw,
            "sinw": sinw,
            "mask": np.ascontiguousarray(mask),
        })
    return in_maps


def kernel(x, positions, Wq, bq, Wk, bk, Wv, bv, Wo, bo):
    global LAST_RESULT
    from concourse.bass_utils import run_bass_kernel_spmd

    x = np.asarray(x, dtype=np.float32)
    positions = np.asarray(positions)
    args = [np.asarray(a, dtype=np.float32) for a in (Wq, bq, Wk, bk, Wv, bv, Wo, bo)]
    Wq, bq, Wk, bk, Wv, bv, Wo, bo = args

    nc = _get_nc()
    in_maps = _host_prep(x, positions, Wq, bq, Wk, bk, Wv, bv, Wo, bo)
    try:
        res = run_bass_kernel_spmd(nc, in_maps, core_ids=list(range(NCORES)))
    except ModuleNotFoundError:
        # axon NTFF profiling hook unavailable in this image; run untraced
        os.environ["BASS_NEVER_TRACE"] = "1"
        res = run_bass_kernel_spmd(nc, in_maps, core_ids=list(range(NCORES)))
    LAST_RESULT = res

    out = np.empty((B, S, F), dtype=np.float32)
    for b in range(B):
        acc = np.zeros((F, S), dtype=np.float32)
        for g in range(4):
            acc += res.results[4 * b + g]["outT"]
        out[b] = acc.T + bo[None, :]
    return out


# revision 19
# speedup vs baseline: 1.3306x; 1.0796x over previous
"""Multi-head causal attention with RoPE on 8 trn2 cores.

Sharding: core c -> batch b = c // 4, head group g = c % 4 (heads 4g..4g+4).
Each core computes q/k/v projections for its 4 heads, causal attention, and
a partial output-projection (its heads' slice of Wo). The host sums the 4
partials per batch (tensor-parallel unshard) and adds the output bias.

v3 layout/schedule notes:
  - DMA order: wq first, then xT chunks, then wk/wv/rest, so projection
    matmuls start as soon as each xT chunk lands.
  - Wq/Wk columns are permuted so the rotary "x1" halves of all 4 heads form
    output partitions 0..127 and the "x2" halves a second 128 chunk; RoPE is
    6 full-width vector ops per projection chunk.
  - Attention runs per q-macro (512 q) in TWO PASSES of one head-pair each.
    Per kk chunk and pass: scoresT [kk, q] for the 2 heads go to a
    [128, 1024] psum pair-tile (bufs=2 -> next chunk's score matmuls overlap
    this chunk's EXP, keeping ScalarE's exp stream back-to-back). PV uses
    v with an appended ones column (M=65): psum row 64 accumulates the
    softmax denominator, one bank per head. Every psum bank holds exactly
    one matmul accumulation group covering one partition range.
  - Normalize: copy pv bank to SBUF (frees the bank), reciprocal of row 64
    read in place at partition base 64, gpsimd broadcast, one DVE multiply
    writing the pair-stacked attention output directly into aoT layout.
  - Projections for s-chunks 1..3 and the per-macro output projection are
    emitted between macros; they run on 2 dedicated psum banks ("lin",
    bufs=2) and fill TensorE gaps under the exp-bound attention phase.
    All psum drains stay off ScalarE during attention (DVE) so ScalarE
    does nothing but exp.
"""

import os

import numpy as np
import ml_dtypes

BF16 = ml_dtypes.bfloat16

B, S, F = 2, 2048, 1024
H, D = 16, 64
HALF = D // 2
NCORES = 8
HPC = 4  # heads per core
S_TILES = S // 128  # 16
N_CH = S // 512  # 4  (512-wide column chunks of s)
F_CH = F // 128  # 8
MACROS = 4  # q macro tiles of 512
MAX_WAVELENGTH = 10000.0

_CACHE = {}
LAST_RESULT = None


def _build_nc():
    import concourse.bacc as bacc
    import concourse.tile as tile
    import concourse.mybir as mybir

    fp32 = mybir.dt.float32
    bf16 = mybir.dt.bfloat16
    MULT = mybir.AluOpType.mult
    ADD = mybir.AluOpType.add
    EXP = mybir.ActivationFunctionType.Exp

    nc = bacc.Bacc("TRN2", target_bir_lowering=False, debug=False)

    xT_d = nc.dram_tensor("xT", [F, S], bf16, kind="ExternalInput")
    wq_d = nc.dram_tensor("wq", [F, 256], bf16, kind="ExternalInput")
    wk_d = nc.dram_tensor("wk", [F, 256], bf16, kind="ExternalInput")
    wv_d = nc.dram_tensor("wv", [F + 1, 260], bf16, kind="ExternalInput")
    wo_d = nc.dram_tensor("wo", [256, F], bf16, kind="ExternalInput")
    bqk_d = nc.dram_tensor("bqk", [128, 4], fp32, kind="ExternalInput")
    cos_d = nc.dram_tensor("cosw", [128, S], bf16, kind="ExternalInput")
    sin_d = nc.dram_tensor("sinw", [128, S], bf16, kind="ExternalInput")
    mask_d = nc.dram_tensor("mask", [128, 256], bf16, kind="ExternalInput")
    outT_d = nc.dram_tensor("outT", [F, S], fp32, kind="ExternalOutput")

    with tile.TileContext(nc) as tc:
        with (
            tc.tile_pool(name="persist", bufs=1) as persist,
            tc.tile_pool(name="tmp", bufs=8) as tmp,
            tc.tile_pool(name="attn", bufs=3) as attn_pool,
            tc.tile_pool(name="nrm", bufs=4) as nrm,
            tc.tile_pool(name="ostage", bufs=4) as ostage,
            tc.tile_pool(name="psSC", bufs=2, space="PSUM") as psSC,
            tc.tile_pool(name="psPV", bufs=2, space="PSUM") as psPV,
            tc.tile_pool(name="psLin", bufs=2, space="PSUM") as psLin,
        ):
            # ---------------- persistent SBUF tensors + loads ----------
            # load order = DMA issue order: wq, xT (q-proj can start), wk,
            # wv, then the small stuff and wo (needed last).
            bqk = persist.tile([128, 4], fp32, tag="bqk", name="bqk")
            nc.sync.dma_start(out=bqk, in_=bqk_d[:, :])
            wq = [persist.tile([128, 256], bf16, tag=f"wq{i}", name=f"wq{i}") for i in range(F_CH)]
            for i in range(F_CH):
                nc.sync.dma_start(out=wq[i], in_=wq_d[128 * i : 128 * (i + 1), :])
            xT = [persist.tile([128, S], bf16, tag=f"xT{i}", name=f"xT{i}") for i in range(F_CH)]
            for i in range(F_CH):
                nc.sync.dma_start(out=xT[i], in_=xT_d[128 * i : 128 * (i + 1), :])
            wk = [persist.tile([128, 256], bf16, tag=f"wk{i}", name=f"wk{i}") for i in range(F_CH)]
            for i in range(F_CH):
                nc.sync.dma_start(out=wk[i], in_=wk_d[128 * i : 128 * (i + 1), :])
            wv = [persist.tile([128, 260], bf16, tag=f"wv{i}", name=f"wv{i}") for i in range(F_CH)]
            for i in range(F_CH):
                nc.sync.dma_start(out=wv[i], in_=wv_d[128 * i : 128 * (i + 1), :])
            wvb = persist.tile([1, 260], bf16, tag="wvb", name="wvb")
            nc.sync.dma_start(out=wvb, in_=wv_d[F : F + 1, :])
            cosw = persist.tile([128, S], bf16, tag="cosw", name="cosw")
            sinw = persist.tile([128, S], bf16, tag="sinw", name="sinw")
            nc.sync.dma_start(out=cosw, in_=cos_d[:, :])
            nc.sync.dma_start(out=sinw, in_=sin_d[:, :])
            maskt = persist.tile([128, 256], bf16, tag="maskt", name="maskt")
            nc.sync.dma_start(out=maskt, in_=mask_d[:, :])
            wo = [persist.tile([128, F], bf16, tag=f"wo{i}", name=f"wo{i}") for i in range(2)]
            for i in range(2):
                nc.sync.dma_start(out=wo[i], in_=wo_d[128 * i : 128 * (i + 1), :])

            # post-RoPE q/k, transposed layout [d, s]; chunk 1 = x1 halves
            # of the 4 heads (head h -> partitions 32h..32h+32), chunk 2 = x2.
            q1 = persist.tile([128, S], bf16, tag="q1", name="q1")
            q2 = persist.tile([128, S], bf16, tag="q2", name="q2")
            k1 = persist.tile([128, S], bf16, tag="k1", name="k1")
            k2 = persist.tile([128, S], bf16, tag="k2", name="k2")
            # v in [s, d] layout; head h cols 65h..65h+64, col 65h+64 = ones
            v_sb = [persist.tile([128, 260], bf16, tag=f"v{i}", name=f"v{i}") for i in range(S_TILES)]
            # attention output, [dh, s] pair layout: pair p tile, head 2p at
            # rows 0..63, head 2p+1 at rows 64..127
            aoT = [persist.tile([128, S], bf16, tag=f"aoT{i}", name=f"aoT{i}") for i in range(2)]

            def emit_qk_half(n, w_sb, half, b0, out):
                # one projection half: 8 accumulating matmuls + DVE bias drain
                nsl = slice(512 * n, 512 * (n + 1))
                ps = psLin.tile([128, 512], fp32, tag="lin", name="ps")
                for kc in range(F_CH):
                    nc.tensor.matmul(ps, w_sb[kc][:, 128 * half:128 * half + 128],
                                     xT[kc][:, nsl],
                                     start=(kc == 0), stop=(kc == F_CH - 1))
                nc.vector.tensor_scalar_add(out, ps, bqk[:, b0 + half:b0 + half + 1])

            def emit_rope(n, c1, c2, o1, o2):
                nsl = slice(512 * n, 512 * (n + 1))
                t1 = tmp.tile([128, 512], bf16, tag="rope", name="t1")
                t2 = tmp.tile([128, 512], bf16, tag="rope", name="t2")
                t3 = tmp.tile([128, 512], bf16, tag="rope", name="t3")
                t4 = tmp.tile([128, 512], bf16, tag="rope", name="t4")
                # x1' = x1*cos - x2*sin ; x2' = x2*cos + x1*sin
                nc.vector.tensor_mul(t1, c1, cosw[:, nsl])
                nc.vector.tensor_mul(t2, c2, sinw[:, nsl])
                nc.vector.tensor_mul(t3, c2, cosw[:, nsl])
                nc.vector.tensor_mul(t4, c1, sinw[:, nsl])
                nc.vector.tensor_sub(o1[:, nsl], t1, t2)
                nc.vector.tensor_add(o2[:, nsl], t3, t4)

            def proj_fillers(n):
                # closures, each one psum-group, to interleave between
                # attention chunks (fills PE while ScalarE runs exp)
                fs = []
                for (w_sb, b0, o1, o2) in ((wq, 0, q1, q2), (wk, 2, k1, k2)):
                    c1 = tmp.tile([128, 512], bf16, tag="rope", name="c1")
                    c2 = tmp.tile([128, 512], bf16, tag="rope", name="c2")
                    fs.append(lambda n=n, w=w_sb, c=c1, b=b0: emit_qk_half(n, w, 0, b, c))
                    fs.append(lambda n=n, w=w_sb, c=c2, b=b0: emit_qk_half(n, w, 1, b, c))
                    fs.append(lambda n=n, a=c1, b=c2, u=o1, v=o2: emit_rope(n, a, b, u, v))
                for st in range(4 * n, 4 * n + 4):
                    fs.append(lambda st=st: emit_v_tile(st))
                return fs

            def emit_v_tile(st):
                # v projection for one s-tile; bias via the appended wv bias
                # row (K=1 matmul), drain on DVE
                ps = psLin.tile([128, 260], fp32, tag="lin", name="psv")
                sl = slice(128 * st, 128 * (st + 1))
                for kc in range(F_CH):
                    nc.tensor.matmul(ps, xT[kc][:, sl], wv[kc], start=(kc == 0), stop=False)
                nc.tensor.matmul(ps, xones[:, sl], wvb, start=False, stop=True)
                nc.vector.tensor_copy(v_sb[st], ps)

            def emit_attn_pass(m, p, fillers, chunks_left):
                # heads 2p, 2p+1 of q-macro m; pops fillers AFTER each kk
                # chunk's matmuls (so attention stays ahead in the PE queue)
                # at a rate that spreads them over the remaining chunks
                msl = slice(512 * m, 512 * (m + 1))
                pv = [psPV.tile([65, 512], fp32, tag="pv", name=f"pv{j}") for j in range(2)]
                last = 4 * m + 3
                for kk in range(4 * m + 4):
                    t = kk - 4 * m  # >= 0 -> this kk-chunk holds the diagonal
                    lo = max(0, t) * 128
                    ksl = slice(128 * kk, 128 * (kk + 1))
                    qsl = slice(512 * m + lo, 512 * (m + 1))
                    sps = psSC.tile([128, 1024], fp32, tag="sc", name="sps")
                    for j in range(2):
                        h = 2 * p + j
                        hp = slice(32 * h, 32 * (h + 1))
                        osl = slice(512 * j + lo, 512 * j + 512)
                        nc.tensor.matmul(sps[:, osl], k1[hp, ksl], q1[hp, qsl],
                                         start=True, stop=False, tile_position=(32 * h, 0))
                    for j in range(2):
                        h = 2 * p + j
                        hp = slice(32 * h, 32 * (h + 1))
                        osl = slice(512 * j + lo, 512 * j + 512)
                        nc.tensor.matmul(sps[:, osl], k2[hp, ksl], q2[hp, qsl],
                                         start=False, stop=True, tile_position=(32 * h, 0))
                    at = attn_pool.tile([128, 1024], bf16, tag="attn", name="at")
                    sps_v = sps[:, :].rearrange("a (h q) -> a h q", h=2)[:, :, lo:512]
                    at_v = at[:, :].rearrange("a (h q) -> a h q", h=2)[:, :, lo:512]
                    nc.scalar.activation(out=at_v, in_=sps_v, func=EXP, scale=0.125)
                    if t >= 0:
                        dv = at[:, :].rearrange("a (h q) -> a h q", h=2)[:, :, 128 * t:128 * (t + 1)]
                        mv = maskt[:, :].rearrange("a (h q) -> a h q", h=2)
                        nc.vector.tensor_tensor(dv, dv, mv, op=MULT)
                    for j in range(2):
                        h = 2 * p + j
                        nc.tensor.matmul(
                            pv[j][:, lo:512],
                            v_sb[kk][:, 65 * h:65 * h + 65],
                            at[:, 512 * j + lo:512 * j + 512],
                            start=(kk == 0), stop=(kk == last))
                    npop = -(-len(fillers) // chunks_left[0]) if fillers else 0
                    for _ in range(min(npop, len(fillers))):
                        fillers.pop(0)()
                    chunks_left[0] -= 1
                # normalize: rows 0..63 scaled by 1/row64. DVE ops require a
                # single base partition shared by ALL operands, so the sums
                # row is DMA-hopped to partition 0 and head 2p+1's output
                # goes through a base-0 staging tile + DMA into aoT.
                for j in range(2):
                    s65 = nrm.tile([65, 512], fp32, tag="s65", name="s65")
                    nc.vector.tensor_copy(s65, pv[j][0:65, :])
                    rcpin = nrm.tile([1, 512], fp32, tag="rcpin", name="rcpin")
                    nc.sync.dma_start(out=rcpin, in_=s65[64:65, :])
                    rcp = nrm.tile([1, 512], fp32, tag="rcp", name="rcp")
                    nc.vector.reciprocal_approx_fast(rcp, rcpin)
                    rb = nrm.tile([64, 512], fp32, tag="rb", name="rb")
                    nc.gpsimd.partition_broadcast(rb, rcp[0:1, :])
                    if j == 0:
                        nc.vector.tensor_tensor(aoT[p][0:64, msl],
                                                s65[0:64, :], rb, op=MULT)
                    else:
                        ao64 = nrm.tile([64, 512], bf16, tag="ao64", name="ao64")
                        nc.vector.tensor_tensor(ao64, s65[0:64, :], rb, op=MULT)
                        nc.sync.dma_start(out=aoT[p][64:128, msl], in_=ao64)

            def emit_out_chunk(m, fo):
                msl = slice(512 * m, 512 * (m + 1))
                fsl = slice(128 * fo, 128 * (fo + 1))
                pw = psLin.tile([128, 512], fp32, tag="lin", name="pw")
                for c in range(2):
                    nc.tensor.matmul(pw, wo[c][:, fsl], aoT[c][:, msl],
                                     start=(c == 0), stop=(c == 1))
                ow = ostage.tile([128, 512], fp32, tag="ow", name="ow")
                nc.vector.tensor_copy(ow, pw)
                nc.sync.dma_start(out=outT_d[fsl, msl], in_=ow)

            def out_proj_fillers(m):
                return [lambda m=m, fo=fo: emit_out_chunk(m, fo) for fo in range(F_CH)]

            # ones row of x for the v bias matmul
            xones = persist.tile([1, S], bf16, tag="xones", name="xones")
            nc.vector.memset(xones, 1.0)

            # ---------------- emission schedule ------------------------
            # proj(0) runs up front (gated by the input DMA stream); proj of
            # s-chunk m+1 and out-proj of earlier macros interleave as
            # fillers spread across the attention chunks of macro m (keeps
            # PE dense and HAM-warm while ScalarE streams exps). Out-proj of
            # macro m-2 runs inside macro m so late macros keep filler work.
            for f in proj_fillers(0):
                f()
            # filler assignment per macro: proj of the next s-chunk, plus
            # out-proj of completed macros pushed late so macro 3 stays busy
            extra = {0: [], 1: [0], 2: [], 3: [1, 2]}
            for m in range(MACROS):
                fillers = []
                if m + 1 < MACROS:
                    fillers += proj_fillers(m + 1)
                for mm in extra[m]:
                    fillers += out_proj_fillers(mm)
                chunks_left = [2 * (4 * m + 4)]
                emit_attn_pass(m, 0, fillers, chunks_left)
                emit_attn_pass(m, 1, fillers, chunks_left)
                for f in fillers:  # leftovers before the next macro
                    f()
            for f in out_proj_fillers(MACROS - 1):
                f()

    nc.compile()
    return nc


def _get_nc():
    if "nc" not in _CACHE:
        _CACHE["nc"] = _build_nc()
    return _CACHE["nc"]


def _host_prep(x, positions, Wq, bq, Wk, bk, Wv, bv, Wo, bo):
    """Build the 8 per-core input maps."""
    ts = MAX_WAVELENGTH ** (2.0 * np.arange(HALF, dtype=np.float32) / D)  # [32]
    in_maps = []
    for c in range(NCORES):
        b, g = c // 4, c % 4
        heads = np.arange(4 * g, 4 * g + 4)
        cols_x1 = np.concatenate([64 * h + np.arange(32) for h in heads])
        cols_x2 = cols_x1 + 32
        perm = np.concatenate([cols_x1, cols_x2])

        wv_e = np.zeros((F + 1, 260), dtype=np.float32)
        for hl, h in enumerate(heads):
            wv_e[:F, 65 * hl:65 * hl + 64] = Wv[:, 64 * h:64 * h + 64]
            wv_e[F, 65 * hl:65 * hl + 64] = bv[64 * h:64 * h + 64]
            wv_e[F, 65 * hl + 64] = 1.0

        bqk = np.stack([bq[cols_x1], bq[cols_x2], bk[cols_x1], bk[cols_x2]],
                       axis=1).astype(np.float32)  # [128, 4]

        pos = positions[b].astype(np.float32)  # [S]
        ang = pos[None, :] / ts[:, None]  # [32, S]
        cosw = np.tile(np.cos(ang), (4, 1)).astype(BF16)
        sinw = np.tile(np.sin(ang), (4, 1)).astype(BF16)

        ii = np.arange(128)
        mask = np.tile((ii[:, None] <= ii[None, :]).astype(BF16), (1, 2))

        in_maps.append({
            "xT": np.ascontiguousarray(x[b].T).astype(BF16),
            "wq": Wq[:, perm].astype(BF16),
            "wk": Wk[:, perm].astype(BF16),
            "wv": wv_e.astype(BF16),
            "wo": Wo[64 * heads[0]:64 * heads[0] + 256, :].astype(BF16),
            "bqk": bqk,
            "cosw": cosw,
            "sinw": sinw,
            "mask": np.ascontiguousarray(mask),
        })
    return in_maps


def kernel(x, positions, Wq, bq, Wk, bk, Wv, bv, Wo, bo):
    global LAST_RESULT
    from concourse.bass_utils import run_bass_kernel_spmd

    x = np.asarray(x, dtype=np.float32)
    positions = np.asarray(positions)
    args = [np.asarray(a, dtype=np.float32) for a in (Wq, bq, Wk, bk, Wv, bv, Wo, bo)]
    Wq, bq, Wk, bk, Wv, bv, Wo, bo = args

    nc = _get_nc()
    in_maps = _host_prep(x, positions, Wq, bq, Wk, bk, Wv, bv, Wo, bo)
    try:
        res = run_bass_kernel_spmd(nc, in_maps, core_ids=list(range(NCORES)))
    except ModuleNotFoundError:
        # axon NTFF profiling hook unavailable in this image; run untraced
        os.environ["BASS_NEVER_TRACE"] = "1"
        res = run_bass_kernel_spmd(nc, in_maps, core_ids=list(range(NCORES)))
    LAST_RESULT = res

    out = np.empty((B, S, F), dtype=np.float32)
    for b in range(B):
        acc = np.zeros((F, S), dtype=np.float32)
        for g in range(4):
            acc += res.results[4 * b + g]["outT"]
        out[b] = acc.T + bo[None, :]
    return out


# revision 31
# speedup vs baseline: 1.3694x; 1.0292x over previous
"""Multi-head causal attention with RoPE on 8 trn2 cores.

Sharding: core c -> batch b = c // 4, head group g = c % 4 (heads 4g..4g+4).
Each core computes q/k/v projections for its 4 heads, causal attention, and
a partial output-projection (its heads' slice of Wo). The host sums the 4
partials per batch (tensor-parallel unshard) and adds the output bias.

v3 layout/schedule notes:
  - DMA order: wq first, then xT chunks, then wk/wv/rest, so projection
    matmuls start as soon as each xT chunk lands.
  - Wq/Wk columns are permuted so the rotary "x1" halves of all 4 heads form
    output partitions 0..127 and the "x2" halves a second 128 chunk; RoPE is
    6 full-width vector ops per projection chunk.
  - Attention runs per q-macro (512 q) in TWO PASSES of one head-pair each.
    Per kk chunk and pass: scoresT [kk, q] for the 2 heads go to a
    [128, 1024] psum pair-tile (bufs=2 -> next chunk's score matmuls overlap
    this chunk's EXP, keeping ScalarE's exp stream back-to-back). PV uses
    v with an appended ones column (M=65): psum row 64 accumulates the
    softmax denominator, one bank per head. Every psum bank holds exactly
    one matmul accumulation group covering one partition range.
  - Normalize: copy pv bank to SBUF (frees the bank), reciprocal of row 64
    read in place at partition base 64, gpsimd broadcast, one DVE multiply
    writing the pair-stacked attention output directly into aoT layout.
  - Projections for s-chunks 1..3 and the per-macro output projection are
    emitted between macros; they run on 2 dedicated psum banks ("lin",
    bufs=2) and fill TensorE gaps under the exp-bound attention phase.
    All psum drains stay off ScalarE during attention (DVE) so ScalarE
    does nothing but exp.
"""

import os

import numpy as np
import ml_dtypes

BF16 = ml_dtypes.bfloat16

B, S, F = 2, 2048, 1024
H, D = 16, 64
HALF = D // 2
NCORES = 8
HPC = 4  # heads per core
S_TILES = S // 128  # 16
N_CH = S // 512  # 4  (512-wide column chunks of s)
F_CH = F // 128  # 8
MACROS = 4  # q macro tiles of 512
MAX_WAVELENGTH = 10000.0

_CACHE = {}
LAST_RESULT = None


def _build_nc():
    import concourse.bacc as bacc
    import concourse.tile as tile
    import concourse.mybir as mybir

    fp32 = mybir.dt.float32
    bf16 = mybir.dt.bfloat16
    MULT = mybir.AluOpType.mult
    ADD = mybir.AluOpType.add
    EXP = mybir.ActivationFunctionType.Exp

    nc = bacc.Bacc("TRN2", target_bir_lowering=False, debug=False)

    xT_d = nc.dram_tensor("xT", [F, S], bf16, kind="ExternalInput")
    wq_d = nc.dram_tensor("wq", [F, 256], bf16, kind="ExternalInput")
    wk_d = nc.dram_tensor("wk", [F, 256], bf16, kind="ExternalInput")
    wv_d = nc.dram_tensor("wv", [F, 260], bf16, kind="ExternalInput")
    vb_d = nc.dram_tensor("vb", [128, 260], fp32, kind="ExternalInput")
    wo_d = nc.dram_tensor("wo", [256, F], bf16, kind="ExternalInput")
    bqk_d = nc.dram_tensor("bqk", [128, 4], fp32, kind="ExternalInput")
    cos_d = nc.dram_tensor("cosw", [128, S], bf16, kind="ExternalInput")
    sin_d = nc.dram_tensor("sinw", [128, S], bf16, kind="ExternalInput")
    mask_d = nc.dram_tensor("mask", [128, 256], bf16, kind="ExternalInput")
    outT_d = nc.dram_tensor("outT", [F, S], fp32, kind="ExternalOutput")

    with tile.TileContext(nc) as tc:
        with (
            tc.tile_pool(name="persist", bufs=1) as persist,
            tc.tile_pool(name="tmp", bufs=8) as tmp,
            tc.tile_pool(name="attn", bufs=3) as attn_pool,
            tc.tile_pool(name="nrm", bufs=4) as nrm,
            tc.tile_pool(name="ostage", bufs=4) as ostage,
            tc.tile_pool(name="psSC", bufs=2, space="PSUM") as psSC,
            tc.tile_pool(name="psPV", bufs=2, space="PSUM") as psPV,
            tc.tile_pool(name="psLin", bufs=2, space="PSUM") as psLin,
        ):
            # ---------------- PE warmup ---------------------------------
            # ~50 junk matmuls reading a memset tile run while the input
            # DMAs stream in (PE would otherwise idle); they push the HAM
            # activity window so the PE clock is at 2.4 GHz when the real
            # projection matmuls start.
            xones = persist.tile([1, S], bf16, tag="xones", name="xones")
            nc.vector.memset(xones, 1.0)
            wu = psSC.tile([128, 512], fp32, tag="sc", name="wu")
            for _ in range(48):
                nc.tensor.matmul(wu, xones[0:1, 0:128], xones[0:1, 0:512],
                                 start=True, stop=True)

            # ---------------- persistent SBUF tensors + loads ----------
            # load order = DMA issue order: wq, xT (q-proj can start), wk,
            # wv, then the small stuff and wo (needed last).
            bqk = persist.tile([128, 4], fp32, tag="bqk", name="bqk")
            nc.sync.dma_start(out=bqk, in_=bqk_d[:, :])
            wq = [persist.tile([128, 256], bf16, tag=f"wq{i}", name=f"wq{i}") for i in range(F_CH)]
            for i in range(F_CH):
                nc.sync.dma_start(out=wq[i], in_=wq_d[128 * i : 128 * (i + 1), :])
            xT = [persist.tile([128, S], bf16, tag=f"xT{i}", name=f"xT{i}") for i in range(F_CH)]
            for i in range(F_CH):
                nc.sync.dma_start(out=xT[i], in_=xT_d[128 * i : 128 * (i + 1), :])
            wk = [persist.tile([128, 256], bf16, tag=f"wk{i}", name=f"wk{i}") for i in range(F_CH)]
            for i in range(F_CH):
                nc.sync.dma_start(out=wk[i], in_=wk_d[128 * i : 128 * (i + 1), :])
            wv = [persist.tile([128, 260], bf16, tag=f"wv{i}", name=f"wv{i}") for i in range(F_CH)]
            for i in range(F_CH):
                nc.sync.dma_start(out=wv[i], in_=wv_d[128 * i : 128 * (i + 1), :])
            vbias = persist.tile([128, 260], fp32, tag="vbias", name="vbias")
            nc.sync.dma_start(out=vbias, in_=vb_d[:, :])
            cosw = persist.tile([128, S], bf16, tag="cosw", name="cosw")
            sinw = persist.tile([128, S], bf16, tag="sinw", name="sinw")
            nc.sync.dma_start(out=cosw, in_=cos_d[:, :])
            nc.sync.dma_start(out=sinw, in_=sin_d[:, :])
            maskt = persist.tile([128, 256], bf16, tag="maskt", name="maskt")
            nc.sync.dma_start(out=maskt, in_=mask_d[:, :])
            wo = [persist.tile([128, F], bf16, tag=f"wo{i}", name=f"wo{i}") for i in range(2)]
            for i in range(2):
                nc.sync.dma_start(out=wo[i], in_=wo_d[128 * i : 128 * (i + 1), :])

            # post-RoPE q/k, transposed layout [d, s]; chunk 1 = x1 halves
            # of the 4 heads (head h -> partitions 32h..32h+32), chunk 2 = x2.
            q1 = persist.tile([128, S], bf16, tag="q1", name="q1")
            q2 = persist.tile([128, S], bf16, tag="q2", name="q2")
            k1 = persist.tile([128, S], bf16, tag="k1", name="k1")
            k2 = persist.tile([128, S], bf16, tag="k2", name="k2")
            # v in [s, d] layout; head h cols 65h..65h+64, col 65h+64 = ones
            v_sb = [persist.tile([128, 260], bf16, tag=f"v{i}", name=f"v{i}") for i in range(S_TILES)]
            # attention output, [dh, s] pair layout: pair p tile, head 2p at
            # rows 0..63, head 2p+1 at rows 64..127
            aoT = [persist.tile([128, S], bf16, tag=f"aoT{i}", name=f"aoT{i}") for i in range(2)]

            def emit_qk_half(n, w_sb, half, b0, out):
                # one projection half: 8 accumulating matmuls + DVE bias drain
                nsl = slice(512 * n, 512 * (n + 1))
                ps = psLin.tile([128, 512], fp32, tag="lin", name="ps")
                for kc in range(F_CH):
                    nc.tensor.matmul(ps, w_sb[kc][:, 128 * half:128 * half + 128],
                                     xT[kc][:, nsl],
                                     start=(kc == 0), stop=(kc == F_CH - 1))
                nc.vector.tensor_scalar_add(out, ps, bqk[:, b0 + half:b0 + half + 1])

            def emit_rope(n, c1, c2, o1, o2):
                nsl = slice(512 * n, 512 * (n + 1))
                t1 = tmp.tile([128, 512], bf16, tag="rope", name="t1")
                t2 = tmp.tile([128, 512], bf16, tag="rope", name="t2")
                t3 = tmp.tile([128, 512], bf16, tag="rope", name="t3")
                t4 = tmp.tile([128, 512], bf16, tag="rope", name="t4")
                # x1' = x1*cos - x2*sin ; x2' = x2*cos + x1*sin
                nc.vector.tensor_mul(t1, c1, cosw[:, nsl])
                nc.vector.tensor_mul(t2, c2, sinw[:, nsl])
                nc.vector.tensor_mul(t3, c2, cosw[:, nsl])
                nc.vector.tensor_mul(t4, c1, sinw[:, nsl])
                nc.vector.tensor_sub(o1[:, nsl], t1, t2)
                nc.vector.tensor_add(o2[:, nsl], t3, t4)

            def proj_fillers(n):
                # closures, each one psum-group, to interleave between
                # attention chunks (fills PE while ScalarE runs exp)
                fs = []
                for (w_sb, b0, o1, o2) in ((wq, 0, q1, q2), (wk, 2, k1, k2)):
                    c1 = tmp.tile([128, 512], bf16, tag="rope", name="c1")
                    c2 = tmp.tile([128, 512], bf16, tag="rope", name="c2")
                    fs.append(lambda n=n, w=w_sb, c=c1, b=b0: emit_qk_half(n, w, 0, b, c))
                    fs.append(lambda n=n, w=w_sb, c=c2, b=b0: emit_qk_half(n, w, 1, b, c))
                    fs.append(lambda n=n, a=c1, b=c2, u=o1, v=o2: emit_rope(n, a, b, u, v))
                for st in range(4 * n, 4 * n + 4):
                    fs.append(lambda st=st: emit_v_tile(st))
                return fs

            def emit_v_tile(st):
                # v projection for one s-tile; bias + the ones column fused
                # into the DVE drain (vbias has 1.0 at each head's col 64)
                ps = psLin.tile([128, 260], fp32, tag="lin", name="psv")
                sl = slice(128 * st, 128 * (st + 1))
                for kc in range(F_CH):
                    nc.tensor.matmul(ps, xT[kc][:, sl], wv[kc],
                                     start=(kc == 0), stop=(kc == F_CH - 1))
                nc.vector.tensor_tensor(v_sb[st], ps, vbias, op=ADD)

            def emit_attn_pass(m, p, fillers, sched):
                # heads 2p, 2p+1 of q-macro m; pops fillers AFTER each kk
                # chunk's matmuls (so attention stays ahead in the PE queue),
                # spread evenly across the macro's chunks
                msl = slice(512 * m, 512 * (m + 1))
                pv = [psPV.tile([65, 512], fp32, tag="pv", name=f"pv{j}") for j in range(2)]
                last = 4 * m + 3
                for kk in range(4 * m + 4):
                    t = kk - 4 * m  # >= 0 -> this kk-chunk holds the diagonal
                    lo = max(0, t) * 128
                    ksl = slice(128 * kk, 128 * (kk + 1))
                    qsl = slice(512 * m + lo, 512 * (m + 1))
                    sps = psSC.tile([128, 1024], fp32, tag="sc", name="sps")
                    for j in range(2):
                        h = 2 * p + j
                        hp = slice(32 * h, 32 * (h + 1))
                        osl = slice(512 * j + lo, 512 * j + 512)
                        nc.tensor.matmul(sps[:, osl], k1[hp, ksl], q1[hp, qsl],
                                         start=True, stop=False, tile_position=(32 * h, 0))
                    for j in range(2):
                        h = 2 * p + j
                        hp = slice(32 * h, 32 * (h + 1))
                        osl = slice(512 * j + lo, 512 * j + 512)
                        nc.tensor.matmul(sps[:, osl], k2[hp, ksl], q2[hp, qsl],
                                         start=False, stop=True, tile_position=(32 * h, 0))
                    at = attn_pool.tile([128, 1024], bf16, tag="attn", name="at")
                    sps_v = sps[:, :].rearrange("a (h q) -> a h q", h=2)[:, :, lo:512]
                    at_v = at[:, :].rearrange("a (h q) -> a h q", h=2)[:, :, lo:512]
                    nc.scalar.activation(out=at_v, in_=sps_v, func=EXP, scale=0.125)
                    if t >= 0:
                        dv = at[:, :].rearrange("a (h q) -> a h q", h=2)[:, :, 128 * t:128 * (t + 1)]
                        mv = maskt[:, :].rearrange("a (h q) -> a h q", h=2)
                        nc.vector.tensor_tensor(dv, dv, mv, op=MULT)
                    for j in range(2):
                        h = 2 * p + j
                        nc.tensor.matmul(
                            pv[j][:, lo:512],
                            v_sb[kk][:, 65 * h:65 * h + 65],
                            at[:, 512 * j + lo:512 * j + 512],
                            start=(kk == 0), stop=(kk == last))
                    sched["done"] += 1
                    target = (sched["total_f"] * sched["done"]) // sched["total_c"]
                    while fillers and sched["popped"] < target:
                        fillers.pop(0)()
                        sched["popped"] += 1
                # normalize: rows 0..63 scaled by 1/row64. DVE ops require a
                # single base partition shared by ALL operands, so the sums
                # row is DMA-hopped to partition 0 and head 2p+1's output
                # goes through a base-0 staging tile + DMA into aoT.
                for j in range(2):
                    s65 = nrm.tile([65, 512], fp32, tag="s65", name="s65")
                    nc.vector.tensor_copy(s65, pv[j][0:65, :])
                    rcpin = nrm.tile([1, 512], fp32, tag="rcpin", name="rcpin")
                    nc.sync.dma_start(out=rcpin, in_=s65[64:65, :])
                    rcp = nrm.tile([1, 512], fp32, tag="rcp", name="rcp")
                    nc.vector.reciprocal_approx_fast(rcp, rcpin)
                    rb = nrm.tile([64, 512], fp32, tag="rb", name="rb")
                    nc.gpsimd.partition_broadcast(rb, rcp[0:1, :])
                    if j == 0:
                        nc.vector.tensor_tensor(aoT[p][0:64, msl],
                                                s65[0:64, :], rb, op=MULT)
                    else:
                        ao64 = nrm.tile([64, 512], bf16, tag="ao64", name="ao64")
                        nc.vector.tensor_tensor(ao64, s65[0:64, :], rb, op=MULT)
                        nc.sync.dma_start(out=aoT[p][64:128, msl], in_=ao64)

            def emit_out_chunk(m, fo):
                msl = slice(512 * m, 512 * (m + 1))
                fsl = slice(128 * fo, 128 * (fo + 1))
                pw = psLin.tile([128, 512], fp32, tag="lin", name="pw")
                for c in range(2):
                    nc.tensor.matmul(pw, wo[c][:, fsl], aoT[c][:, msl],
                                     start=(c == 0), stop=(c == 1))
                ow = ostage.tile([128, 512], fp32, tag="ow", name="ow")
                nc.vector.tensor_copy(ow, pw)
                nc.sync.dma_start(out=outT_d[fsl, msl], in_=ow)

            def out_proj_fillers(m):
                return [lambda m=m, fo=fo: emit_out_chunk(m, fo) for fo in range(F_CH)]

            # split out-proj for the LAST macro: pair-0 half runs during
            # pass 1 (staged to SBUF), pair-1 half + add + store at the tail
            ow0 = [persist.tile([128, 512], fp32, tag=f"ow0_{fo}", name=f"ow0_{fo}")
                   for fo in range(F_CH)]

            def emit_out_half0(m, fo):
                pw = psLin.tile([128, 512], fp32, tag="lin", name="pw")
                nc.tensor.matmul(pw, wo[0][:, 128 * fo:128 * fo + 128],
                                 aoT[0][:, 512 * m:512 * (m + 1)],
                                 start=True, stop=True)
                nc.vector.tensor_copy(ow0[fo], pw)

            def emit_out_half1(m, fo):
                msl = slice(512 * m, 512 * (m + 1))
                fsl = slice(128 * fo, 128 * (fo + 1))
                pw = psLin.tile([128, 512], fp32, tag="lin", name="pw")
                nc.tensor.matmul(pw, wo[1][:, fsl], aoT[1][:, msl],
                                 start=True, stop=True)
                ow = ostage.tile([128, 512], fp32, tag="ow", name="ow")
                nc.vector.tensor_tensor(ow, pw, ow0[fo], op=ADD)
                nc.sync.dma_start(out=outT_d[fsl, msl], in_=ow)

            # ---------------- emission schedule ------------------------
            # proj(0) runs up front (gated by the input DMA stream); proj of
            # s-chunk m+1 and out-proj of earlier macros interleave as
            # fillers spread across the attention chunks of macro m (keeps
            # PE dense and HAM-warm while ScalarE streams exps). Out-proj of
            # macro m-2 runs inside macro m so late macros keep filler work.
            for f in proj_fillers(0):
                f()
            # filler assignment per macro: proj of the next s-chunk, plus
            # out-proj of completed macros pushed late so macro 3 stays busy
            extra = {0: [], 1: [0], 2: [], 3: [1, 2]}
            mlast = MACROS - 1
            for m in range(MACROS):
                fillers = []
                if m + 1 < MACROS:
                    fillers += proj_fillers(m + 1)
                for mm in extra[m]:
                    fillers += out_proj_fillers(mm)
                nf1 = F_CH if m == mlast else 0  # pass-1 gains the half0 work
                sched = {"done": 0, "popped": 0, "total_c": 2 * (4 * m + 4),
                         "total_f": len(fillers) + nf1}
                emit_attn_pass(m, 0, fillers, sched)
                if m == mlast:
                    fillers += [lambda fo=fo: emit_out_half0(mlast, fo)
                                for fo in range(F_CH)]
                emit_attn_pass(m, 1, fillers, sched)
                for f in fillers:  # leftovers before the next macro
                    f()
            for fo in range(F_CH):
                emit_out_half1(mlast, fo)

    nc.compile()
    return nc


def _get_nc():
    if "nc" not in _CACHE:
        _CACHE["nc"] = _build_nc()
    return _CACHE["nc"]


def _host_prep(x, positions, Wq, bq, Wk, bk, Wv, bv, Wo, bo):
    """Build the 8 per-core input maps."""
    ts = MAX_WAVELENGTH ** (2.0 * np.arange(HALF, dtype=np.float32) / D)  # [32]
    in_maps = []
    for c in range(NCORES):
        b, g = c // 4, c % 4
        heads = np.arange(4 * g, 4 * g + 4)
        cols_x1 = np.concatenate([64 * h + np.arange(32) for h in heads])
        cols_x2 = cols_x1 + 32
        perm = np.concatenate([cols_x1, cols_x2])

        wv_e = np.zeros((F, 260), dtype=np.float32)
        vb_row = np.zeros((260,), dtype=np.float32)
        for hl, h in enumerate(heads):
            wv_e[:, 65 * hl:65 * hl + 64] = Wv[:, 64 * h:64 * h + 64]
            vb_row[65 * hl:65 * hl + 64] = bv[64 * h:64 * h + 64]
            vb_row[65 * hl + 64] = 1.0

        bqk = np.stack([bq[cols_x1], bq[cols_x2], bk[cols_x1], bk[cols_x2]],
                       axis=1).astype(np.float32)  # [128, 4]

        pos = positions[b].astype(np.float32)  # [S]
        ang = pos[None, :] / ts[:, None]  # [32, S]
        cosw = np.tile(np.cos(ang), (4, 1)).astype(BF16)
        sinw = np.tile(np.sin(ang), (4, 1)).astype(BF16)

        ii = np.arange(128)
        mask = np.tile((ii[:, None] <= ii[None, :]).astype(BF16), (1, 2))

        in_maps.append({
            "xT": np.ascontiguousarray(x[b].T).astype(BF16),
            "wq": Wq[:, perm].astype(BF16),
            "wk": Wk[:, perm].astype(BF16),
            "wv": wv_e.astype(BF16),
            "vb": np.tile(vb_row[None, :], (128, 1)).astype(np.float32),
            "wo": Wo[64 * heads[0]:64 * heads[0] + 256, :].astype(BF16),
            "bqk": bqk,
            "cosw": cosw,
            "sinw": sinw,
            "mask": np.ascontiguousarray(mask),
        })
    return in_maps


def kernel(x, positions, Wq, bq, Wk, bk, Wv, bv, Wo, bo):
    global LAST_RESULT
    from concourse.bass_utils import run_bass_kernel_spmd

    x = np.asarray(x, dtype=np.float32)
    positions = np.asarray(positions)
    args = [np.asarray(a, dtype=np.float32) for a in (Wq, bq, Wk, bk, Wv, bv, Wo, bo)]
    Wq, bq, Wk, bk, Wv, bv, Wo, bo = args

    nc = _get_nc()
    in_maps = _host_prep(x, positions, Wq, bq, Wk, bk, Wv, bv, Wo, bo)
    try:
        res = run_bass_kernel_spmd(nc, in_maps, core_ids=list(range(NCORES)))
    except ModuleNotFoundError:
        # axon NTFF profiling hook unavailable in this image; run untraced
        os.environ["BASS_NEVER_TRACE"] = "1"
        res = run_bass_kernel_spmd(nc, in_maps, core_ids=list(range(NCORES)))
    LAST_RESULT = res

    out = np.empty((B, S, F), dtype=np.float32)
    for b in range(B):
        acc = np.zeros((F, S), dtype=np.float32)
        for g in range(4):
            acc += res.results[4 * b + g]["outT"]
        out[b] = acc.T + bo[None, :]
    return out
